# revision 13
# baseline (speedup 1.0000x reference)
"""CenterLoss kernel for 8 Trainium2 NeuronCores.

reference:
    w_t = weight[targets]                    # [N, D] gather
    d   = sqrt(sum((x - w_t)^2, axis=1) + 1e-6)
    out = mean(d)

Strategy (data-parallel over N, class-sorted layout):
  - Host sorts rows by target class and ships each core a shard laid
    out so device row-group g (= partition-p slot t=g of the SBUF
    tile) holds 128 *consecutive* sorted rows.  128 consecutive
    sorted rows span at most ~4 distinct classes (N/C = 65.5 rows per
    class), so the per-row center fetch collapses to one tiny
    contract-8 matmul per row-group: a host-built [8, 128] one-hot
    against a per-group 8-class slab of the center table.  All
    selection data for the whole core is ~0.6 MB, one upfront DMA --
    HBM traffic is essentially just the x stream (16.8 MB/core).
  - Expansion s = ||x||^2 - 2 x.w + ||w||^2: ACT square+accumulates x
    for most row-groups (797 ns each incl. accumulator read), DVE does
    the fused x*w multiply+row-accumulate for every row-group against
    the PSUM centers (658 ns) plus the remaining squares, balancing
    both engines just under the x-stream DMA time.  The host adds
    ||w||^2 (it knows targets).
  - Host: combine partials, sqrt(s + eps), mean over N (<0.01% of the
    FLOPs).  Row order is irrelevant to the mean, so the sorted
    permutation is never undone.
"""

import numpy as np
import ml_dtypes

import concourse.bacc as bacc
import concourse.bass as bass
import concourse.mybir as mybir
from concourse.bass_utils import run_bass_kernel_spmd
from concourse.tile import TileContext

N, D, C = 65536, 512, 1000
NCORES = 8
NSH = N // NCORES            # 8192 rows per core
P = 128
NRG = NSH // P               # 64 row-groups per core
SLAB = 8                     # classes per row-group slab (max span ~4)
PIECE = 8                    # row-groups per sel/oh DMA piece
NPIECE = NRG // PIECE
# x-chunk schedule: small chunks first (first compute starts as soon as
# 512 KB lands) and last (short pipeline drain), big in the middle.
CHUNKS = [2, 2, 4] + [8] * 6 + [4, 2, 2]
assert sum(CHUNKS) == NRG
# row-groups whose ||x||^2 runs on DVE instead of ACT (ACT's accumulator
# read costs 187 ns/op vs DVE's 8 ns; a 55/9 split balances the engines
# at ~44 us each)
DVE_SQ = frozenset(g for g in range(NRG) if g % 7 == 3)
EPS = 1e-6

_dt = mybir.dt


def _build_bass() -> bass.Bass:
    nc = bacc.Bacc(trn_type="TRN2")
    x_d = nc.dram_tensor("x", [NSH, D], _dt.float32, kind="ExternalInput")
    slab_d = nc.dram_tensor("slab", [SLAB, NRG * D], _dt.float8e4, kind="ExternalInput")
    oh_d = nc.dram_tensor("oh", [SLAB, NRG * P], _dt.float8e4, kind="ExternalInput")
    out_d = nc.dram_tensor("out", [P, NRG], _dt.float32, kind="ExternalOutput")
    xx_d = nc.dram_tensor("xx", [P, NRG], _dt.float32, kind="ExternalOutput")

    # device row p*NRG + g  <->  sorted row g*128 + p (host pre-transposes),
    # so partition lines stay 2 KB-contiguous runs
    x_v = x_d[:, :].rearrange("(p t) d -> p t d", p=P)
    slab_v = slab_d[:, :].rearrange("s (g d) -> s g d", g=NRG)
    oh_v = oh_d[:, :].rearrange("s (g r) -> s g r", g=NRG)

    with TileContext(nc) as tc:
        with (
            tc.tile_pool(name="xin", bufs=5) as x_pool,
            tc.tile_pool(name="sqa", bufs=4) as sqa_pool,
            tc.tile_pool(name="sqv", bufs=2) as sqv_pool,
            tc.tile_pool(name="xw", bufs=4) as xw_pool,
            tc.tile_pool(name="psum", bufs=8, space="PSUM") as psum_pool,
            tc.tile_pool(name="small", bufs=1) as small,
        ):
            sel = small.tile([SLAB, NRG, D], _dt.float8e4)
            oht = small.tile([SLAB, NRG, P], _dt.float8e4)
            ssq = small.tile([P, NRG], _dt.float32)
            xxt = small.tile([P, NRG], _dt.float32)

            # sel/oh land in 8-row-group pieces (4 KB descriptors, never a
            # monster 8-partition transfer) on the otherwise-idle GPSIMD
            # DGE ring, keeping the sync ring purely for the x stream.
            for pc in range(NPIECE):
                s = pc * PIECE
                nc.gpsimd.dma_start(
                    out=sel[:, s : s + PIECE, :], in_=slab_v[:, s : s + PIECE, :]
                )
                nc.gpsimd.dma_start(
                    out=oht[:, s : s + PIECE, :], in_=oh_v[:, s : s + PIECE, :]
                )

            g0 = 0
            for ct in CHUNKS:
                x_t = x_pool.tile([P, ct, D], _dt.float32, tag="x")
                nc.sync.dma_start(out=x_t[:], in_=x_v[:, g0 : g0 + ct, :])
                for t in range(ct):
                    g = g0 + t
                    ps = psum_pool.tile([P, D], _dt.float32, tag="ps")
                    nc.tensor.matmul(
                        out=ps[:],
                        lhsT=oht[:, g, :],
                        rhs=sel[:, g, :],
                        start=True,
                        stop=True,
                    )
                    if g in DVE_SQ:
                        sq_t = sqv_pool.tile([P, D], _dt.bfloat16, tag="sqv")
                        nc.vector.scalar_tensor_tensor(
                            out=sq_t[:],
                            in0=x_t[:, t, :],
                            scalar=0.0,
                            in1=x_t[:, t, :],
                            op0=mybir.AluOpType.bypass,
                            op1=mybir.AluOpType.mult,
                            accum_out=xxt[:, g : g + 1],
                        )
                    else:
                        sq_t = sqa_pool.tile([P, D], _dt.bfloat16, tag="sqa")
                        nc.scalar.activation(
                            out=sq_t[:],
                            in_=x_t[:, t, :],
                            func=mybir.ActivationFunctionType.Square,
                            accum_out=xxt[:, g : g + 1],
                        )
                    xw_t = xw_pool.tile([P, D], _dt.bfloat16, tag="xw")
                    nc.vector.scalar_tensor_tensor(
                        out=xw_t[:],
                        in0=x_t[:, t, :],
                        scalar=0.0,
                        in1=ps[:],
                        op0=mybir.AluOpType.bypass,
                        op1=mybir.AluOpType.mult,
                        accum_out=ssq[:, g : g + 1],
                    )
                g0 += ct

            nc.sync.dma_start(out=out_d[:, :], in_=ssq[:])
            nc.sync.dma_start(out=xx_d[:, :], in_=xxt[:])
    nc.finalize()
    return nc


_NC_CACHE = None


def kernel(x, weight, targets):
    global _NC_CACHE
    x = np.ascontiguousarray(np.asarray(x, dtype=np.float32))
    weight = np.ascontiguousarray(np.asarray(weight, dtype=np.float32))
    targets = np.asarray(targets).astype(np.int64)
    assert x.shape == (N, D) and weight.shape == (C, D) and targets.shape == (N,)

    if _NC_CACHE is None:
        _NC_CACHE = _build_bass()
    nc = _NC_CACHE

    order = np.argsort(targets, kind="stable")
    wb = weight.astype(ml_dtypes.float8_e4m3)

    in_maps = []
    tsh_all = []
    for k in range(NCORES):
        rows = order[k * NSH : (k + 1) * NSH]
        tsh = targets[rows].reshape(NRG, P)       # [g, r] sorted classes
        tsh_all.append(tsh)
        c0 = np.minimum(tsh.min(axis=1), C - SLAB)  # [g]
        assert int((tsh.max(axis=1) - c0).max()) < SLAB, "class span > SLAB"
        # slab[s, g, :] = bf16(weight[c0[g] + s])
        slab = np.ascontiguousarray(
            wb[(c0[None, :] + np.arange(SLAB)[:, None])].reshape(SLAB, NRG * D)
        )
        # oh[s, g, r] = 1 iff tsh[g, r] == c0[g] + s
        oh = (
            (tsh[None, :, :] == (c0[None, :] + np.arange(SLAB)[:, None])[:, :, None])
            .astype(ml_dtypes.float8_e4m3)
            .reshape(SLAB, NRG * P)
        )
        # device row p*NRG + g = sorted row g*128 + p
        x_dev = np.ascontiguousarray(
            x[rows].reshape(NRG, P, D).transpose(1, 0, 2).reshape(NSH, D)
        )
        in_maps.append({"x": x_dev, "slab": slab, "oh": np.ascontiguousarray(oh)})

    res = run_bass_kernel_spmd(nc, in_maps, core_ids=list(range(NCORES)))
    wsq = (weight.astype(np.float64) ** 2).sum(1)
    total = np.float64(0.0)
    for k, r in enumerate(res.results):
        xw = r["out"].astype(np.float64)          # [p, g]
        xx = r["xx"].astype(np.float64)
        s = xx - 2.0 * xw + wsq[tsh_all[k]].T     # tsh [g, r] -> [r, g]
        total += np.sqrt(s + EPS).sum()
    return np.float32(total / N)


if __name__ == "__main__":
    rng = np.random.default_rng(0)
    x = rng.standard_normal((N, D), dtype=np.float32)
    w = (rng.standard_normal((C, D)) / np.sqrt(D)).astype(np.float32)
    t = rng.integers(0, C, size=(N,)).astype(np.int64)
    got = kernel(x, w, t)
    wt = w[t]
    exp = np.sqrt(((x - wt) ** 2).sum(1) + EPS).mean()
    print("kernel:", got, "expected:", exp, "rel:", abs(got - exp) / abs(exp))


# revision 14
# speedup vs baseline: 1.0161x; 1.0161x over previous
"""CenterLoss kernel for 8 Trainium2 NeuronCores.

reference:
    w_t = weight[targets]                    # [N, D] gather
    d   = sqrt(sum((x - w_t)^2, axis=1) + 1e-6)
    out = mean(d)

Strategy (data-parallel over N, class-sorted layout):
  - Host sorts rows by target class and ships each core a shard laid
    out so device row-group g (= partition-p slot t=g of the SBUF
    tile) holds 128 *consecutive* sorted rows.  128 consecutive
    sorted rows span at most ~4 distinct classes (N/C = 65.5 rows per
    class), so the per-row center fetch collapses to one tiny
    contract-8 matmul per row-group: a host-built [8, 128] one-hot
    against a per-group 8-class slab of the center table.  All
    selection data for the whole core is ~0.6 MB, one upfront DMA --
    HBM traffic is essentially just the x stream (16.8 MB/core).
  - Expansion s = ||x||^2 - 2 x.w + ||w||^2: ACT square+accumulates x
    for most row-groups (797 ns each incl. accumulator read), DVE does
    the fused x*w multiply+row-accumulate for every row-group against
    the PSUM centers (658 ns) plus the remaining squares, balancing
    both engines just under the x-stream DMA time.  The host adds
    ||w||^2 (it knows targets).
  - Host: combine partials, sqrt(s + eps), mean over N (<0.01% of the
    FLOPs).  Row order is irrelevant to the mean, so the sorted
    permutation is never undone.
"""

import numpy as np
import ml_dtypes

import concourse.bacc as bacc
import concourse.bass as bass
import concourse.mybir as mybir
from concourse.bass_utils import run_bass_kernel_spmd
from concourse.tile import TileContext

N, D, C = 65536, 512, 1000
NCORES = 8
NSH = N // NCORES            # 8192 rows per core
P = 128
NRG = NSH // P               # 64 row-groups per core
SLAB = 8                     # classes per row-group slab (max span ~4)
PIECE = 8                    # row-groups per sel/oh DMA piece
NPIECE = NRG // PIECE
# x-chunk schedule: small chunks first (first compute starts as soon as
# 512 KB lands) and last (short pipeline drain), big in the middle.
CHUNKS = [2, 2, 4] + [8] * 6 + [4, 2, 2]
assert sum(CHUNKS) == NRG
# row-groups whose ||x||^2 runs on DVE instead of ACT (ACT's accumulator
# read costs 187 ns/op vs DVE's 8 ns; a 55/9 split balances the engines
# at ~44 us each)
DVE_SQ = frozenset(g for g in range(NRG) if g % 7 == 3)
EPS = 1e-6

_dt = mybir.dt


def _build_bass() -> bass.Bass:
    nc = bacc.Bacc(trn_type="TRN2")
    x_d = nc.dram_tensor("x", [NSH, D], _dt.float32, kind="ExternalInput")
    slab_d = nc.dram_tensor("slab", [SLAB, NRG * D], _dt.float8e4, kind="ExternalInput")
    oh_d = nc.dram_tensor("oh", [SLAB, NRG * P], _dt.float8e4, kind="ExternalInput")
    out_d = nc.dram_tensor("out", [P, NRG], _dt.float32, kind="ExternalOutput")
    xx_d = nc.dram_tensor("xx", [P, NRG], _dt.float32, kind="ExternalOutput")

    # device row p*NRG + g  <->  sorted row g*128 + p (host pre-transposes),
    # so partition lines stay 2 KB-contiguous runs
    x_v = x_d[:, :].rearrange("(p t) d -> p t d", p=P)
    slab_v = slab_d[:, :].rearrange("s (g d) -> s g d", g=NRG)
    oh_v = oh_d[:, :].rearrange("s (g r) -> s g r", g=NRG)

    with TileContext(nc) as tc:
        with (
            tc.tile_pool(name="xin", bufs=5) as x_pool,
            tc.tile_pool(name="sqa", bufs=4) as sqa_pool,
            tc.tile_pool(name="sqv", bufs=2) as sqv_pool,
            tc.tile_pool(name="xw", bufs=4) as xw_pool,
            tc.tile_pool(name="psum", bufs=8, space="PSUM") as psum_pool,
            tc.tile_pool(name="small", bufs=1) as small,
        ):
            sel = small.tile([SLAB, NRG, D], _dt.float8e4)
            oht = small.tile([SLAB, NRG, P], _dt.float8e4)
            ssq = small.tile([P, NRG], _dt.float32)
            xxt = small.tile([P, NRG], _dt.float32)

            # sel/oh land in 8-row-group pieces (4 KB descriptors, never a
            # monster 8-partition transfer) on the otherwise-idle GPSIMD
            # DGE ring, keeping the sync ring purely for the x stream.
            for pc in range(NPIECE):
                s = pc * PIECE
                # first two pieces ride ahead of chunk 0 on the sync ring
                # (40 KB, ~0.3 us) so the PE can start with the first x
                # chunk; the rest stream on the idle GPSIMD ring.
                eng = nc.sync if pc < 2 else nc.gpsimd
                eng.dma_start(
                    out=sel[:, s : s + PIECE, :], in_=slab_v[:, s : s + PIECE, :]
                )
                eng.dma_start(
                    out=oht[:, s : s + PIECE, :], in_=oh_v[:, s : s + PIECE, :]
                )

            g0 = 0
            for ct in CHUNKS:
                x_t = x_pool.tile([P, ct, D], _dt.float32, tag="x")
                nc.sync.dma_start(out=x_t[:], in_=x_v[:, g0 : g0 + ct, :])
                for t in range(ct):
                    g = g0 + t
                    ps = psum_pool.tile([P, D], _dt.float32, tag="ps")
                    nc.tensor.matmul(
                        out=ps[:],
                        lhsT=oht[:, g, :],
                        rhs=sel[:, g, :],
                        start=True,
                        stop=True,
                    )
                    if g in DVE_SQ:
                        sq_t = sqv_pool.tile([P, D], _dt.bfloat16, tag="sqv")
                        nc.vector.scalar_tensor_tensor(
                            out=sq_t[:],
                            in0=x_t[:, t, :],
                            scalar=0.0,
                            in1=x_t[:, t, :],
                            op0=mybir.AluOpType.bypass,
                            op1=mybir.AluOpType.mult,
                            accum_out=xxt[:, g : g + 1],
                        )
                    else:
                        sq_t = sqa_pool.tile([P, D], _dt.bfloat16, tag="sqa")
                        nc.scalar.activation(
                            out=sq_t[:],
                            in_=x_t[:, t, :],
                            func=mybir.ActivationFunctionType.Square,
                            accum_out=xxt[:, g : g + 1],
                        )
                    xw_t = xw_pool.tile([P, D], _dt.bfloat16, tag="xw")
                    nc.vector.scalar_tensor_tensor(
                        out=xw_t[:],
                        in0=x_t[:, t, :],
                        scalar=0.0,
                        in1=ps[:],
                        op0=mybir.AluOpType.bypass,
                        op1=mybir.AluOpType.mult,
                        accum_out=ssq[:, g : g + 1],
                    )
                g0 += ct

            nc.sync.dma_start(out=out_d[:, :], in_=ssq[:])
            nc.sync.dma_start(out=xx_d[:, :], in_=xxt[:])
    nc.finalize()
    return nc


_NC_CACHE = None


def kernel(x, weight, targets):
    global _NC_CACHE
    x = np.ascontiguousarray(np.asarray(x, dtype=np.float32))
    weight = np.ascontiguousarray(np.asarray(weight, dtype=np.float32))
    targets = np.asarray(targets).astype(np.int64)
    assert x.shape == (N, D) and weight.shape == (C, D) and targets.shape == (N,)

    if _NC_CACHE is None:
        _NC_CACHE = _build_bass()
    nc = _NC_CACHE

    order = np.argsort(targets, kind="stable")
    wb = weight.astype(ml_dtypes.float8_e4m3)

    in_maps = []
    tsh_all = []
    for k in range(NCORES):
        rows = order[k * NSH : (k + 1) * NSH]
        tsh = targets[rows].reshape(NRG, P)       # [g, r] sorted classes
        tsh_all.append(tsh)
        c0 = np.minimum(tsh.min(axis=1), C - SLAB)  # [g]
        assert int((tsh.max(axis=1) - c0).max()) < SLAB, "class span > SLAB"
        # slab[s, g, :] = bf16(weight[c0[g] + s])
        slab = np.ascontiguousarray(
            wb[(c0[None, :] + np.arange(SLAB)[:, None])].reshape(SLAB, NRG * D)
        )
        # oh[s, g, r] = 1 iff tsh[g, r] == c0[g] + s
        oh = (
            (tsh[None, :, :] == (c0[None, :] + np.arange(SLAB)[:, None])[:, :, None])
            .astype(ml_dtypes.float8_e4m3)
            .reshape(SLAB, NRG * P)
        )
        # device row p*NRG + g = sorted row g*128 + p
        x_dev = np.ascontiguousarray(
            x[rows].reshape(NRG, P, D).transpose(1, 0, 2).reshape(NSH, D)
        )
        in_maps.append({"x": x_dev, "slab": slab, "oh": np.ascontiguousarray(oh)})

    res = run_bass_kernel_spmd(nc, in_maps, core_ids=list(range(NCORES)))
    wsq = (weight.astype(np.float64) ** 2).sum(1)
    total = np.float64(0.0)
    for k, r in enumerate(res.results):
        xw = r["out"].astype(np.float64)          # [p, g]
        xx = r["xx"].astype(np.float64)
        s = xx - 2.0 * xw + wsq[tsh_all[k]].T     # tsh [g, r] -> [r, g]
        total += np.sqrt(s + EPS).sum()
    return np.float32(total / N)


if __name__ == "__main__":
    rng = np.random.default_rng(0)
    x = rng.standard_normal((N, D), dtype=np.float32)
    w = (rng.standard_normal((C, D)) / np.sqrt(D)).astype(np.float32)
    t = rng.integers(0, C, size=(N,)).astype(np.int64)
    got = kernel(x, w, t)
    wt = w[t]
    exp = np.sqrt(((x - wt) ** 2).sum(1) + EPS).mean()
    print("kernel:", got, "expected:", exp, "rel:", abs(got - exp) / abs(exp))
